# revision 33
# baseline (speedup 1.0000x reference)
"""Trainium2 Bass kernel for ClassicAttention (B=2, S=2048, D=1024, H=16).

Sharding: tensor-parallel over heads across 8 cores (2 heads/core).
  - Host pre-transposes x to x^T [D, M] and pre-casts all matmul operands
    to bf16, so the kernel has no cast / AllGather / DMA-transpose prologue.
  - QKV projection: each core computes Q^T,K^T (d-major) and V (row-major)
    for its 2 heads over all B*S rows straight from x^T in SBUF.  Biases are
    seeded into PSUM with K=1 outer-product matmuls; evacuation is a plain
    DVE copy (tensor_scalar_add from PSUM measured pathologically slow).
  - Attention: transposed-scores formulation S^T[k,q]; both heads share one
    [128,1024] score tile so each k-tile needs a single exp ACTIVATE.  The
    softmax denominator rides row 64 of the AV accumulator via a ones
    column in V.  Per-kt software pipeline: scores(kt+1) is emitted before
    AV(kt) so the exp(kt) runs while the PE does useful work.  Per-q-group
    normalization broadcasts the sums row with a K=1 ones matmul.
  - c_proj: per-(batch, q-group) AllGather of context (8x 128KB), input
    gathers on the gpsimd DMA queue so AllGather waits never block the
    sync queue; each core computes a 128-column slice of the output,
    transposed ([j, B*S]); the host transposes back.
  - Emission order software-pipelines phases: QKV(b1) matmuls interleave
    into attention(b0)'s PE stream, c_proj units into attention(b1).
All matmuls bf16 inputs with fp32 PSUM accumulation.
"""

import numpy as np
import ml_dtypes

import concourse.bass as bass
import concourse.tile as tile
import concourse.mybir as mybir
from concourse import bacc
from concourse.bass_utils import run_bass_kernel_spmd

F32 = mybir.dt.float32
BF16 = mybir.dt.bfloat16

NCORES = 8
B, S, D = 2, 2048, 1024
H, HD = 16, 64
HPC = H // NCORES          # heads per core = 2
M = B * S                  # 4096 rows
ST_B = S // 128            # 16 s-tiles per batch
KCH = D // 128             # 8 contraction chunks
G_PER_B = S // 512         # 4 q-supers per batch
SCALE = 1.0 / (HD ** 0.5)
EXP = mybir.ActivationFunctionType.Exp
DEBUG = False


def build_ir(nc):
    # ---------------- DRAM I/O ----------------
    xt = nc.dram_tensor("xt", [D, M], BF16, kind="ExternalInput").ap()
    wqk = nc.dram_tensor("wqk", [D, 256], BF16, kind="ExternalInput").ap()
    wv = nc.dram_tensor("wv", [D, 128], BF16, kind="ExternalInput").ap()
    wp = nc.dram_tensor("wp", [D, 128], BF16, kind="ExternalInput").ap()
    bqk = nc.dram_tensor("bqk", [256], F32, kind="ExternalInput").ap()
    biases = nc.dram_tensor("biases", [512], BF16, kind="ExternalInput").ap()
    outT = nc.dram_tensor("outT", [128, M], F32, kind="ExternalOutput").ap()

    # causal mask for the diagonal 128-block: mask[k, c] = 1 if c >= k,
    # duplicated for both heads ([128, 2, 128]) so one mul covers a kt tile
    mask_np = (np.arange(128)[None, :] >= np.arange(128)[:, None])
    mask2 = np.stack([mask_np, mask_np], axis=1)
    mask_const = nc.inline_tensor(mask2.astype(ml_dtypes.bfloat16),
                                  "mask_const").ap()

    rg = [list(range(NCORES))]

    dbg = None
    if DEBUG:
        dbg = {
            "qkt": nc.dram_tensor("dbg_qkt", [128, 2, M], BF16,
                                  kind="ExternalOutput").ap(),
            "v": nc.dram_tensor("dbg_v", [128, B * ST_B, 130], BF16,
                                kind="ExternalOutput").ap(),
            "sums": nc.dram_tensor("dbg_sums", [1, 16, 512], F32,
                                   kind="ExternalOutput").ap(),
            "rec": nc.dram_tensor("dbg_rec", [64, 16, 512], F32,
                                  kind="ExternalOutput").ap(),
            "ctx": nc.dram_tensor("dbg_ctx", [128, 8, 512], BF16,
                                  kind="ExternalOutput").ap(),
        }

    with tile.TileContext(nc) as tc:
        _emit(nc, tc, xt, wqk, wv, wp, bqk, biases, outT, mask_const, rg, dbg)
    return nc


def _emit(nc, tc, xt, wqk, wv, wp, bqk, biases, outT, mask_const, rg, dbg=None):
    import contextlib
    es = contextlib.ExitStack()
    with es:
        singles = es.enter_context(tc.tile_pool(name="singles", bufs=1))
        dram = es.enter_context(tc.tile_pool(name="dram", bufs=1, space="DRAM"))

        # ------------- persistent SBUF -------------
        qT = singles.tile([128, M], BF16, tag="qT")
        kT = singles.tile([128, M], BF16, tag="kT")
        v_sb = singles.tile([128, B * ST_B, 130], BF16, tag="v_sb")
        mask_sb = singles.tile([128, 2, 128], BF16, tag="mask_sb")
        nc.sync.dma_start(out=mask_sb, in_=mask_const)
        nc.vector.memset(v_sb, 1.0)                # ones columns pre-set

        # weights (already bf16 from host)
        wqk_sb = singles.tile([128, KCH, 256], BF16, tag="wqk_sb")
        wv_sb = singles.tile([128, KCH, 128], BF16, tag="wv_sb")
        wp_sb = singles.tile([128, KCH, 128], BF16, tag="wp_sb")
        bqk_sb = singles.tile([128, 2], F32, tag="bqk_sb")
        # bias_row: [bqk(256) | bv(128) | bp(128)] bf16, for K=1 seed matmuls
        bias_row = singles.tile([1, 512], BF16, tag="bias_row")
        ones512 = singles.tile([1, 512], BF16, tag="ones512")
        ones_p64 = singles.tile([65, 64], F32, tag="ones_p64")
        nc.vector.memset(ones512, 1.0)
        nc.vector.memset(ones_p64, 1.0)
        nc.sync.dma_start(out=wqk_sb, in_=wqk.rearrange("(c p) j -> p c j", p=128))
        nc.sync.dma_start(out=wv_sb, in_=wv.rearrange("(c p) j -> p c j", p=128))
        nc.sync.dma_start(out=wp_sb, in_=wp.rearrange("(c p) j -> p c j", p=128))
        nc.sync.dma_start(out=bqk_sb, in_=bqk.rearrange("(t p) -> p t", p=128))
        nc.sync.dma_start(out=bias_row, in_=biases.rearrange("(a j) -> a j", a=1))

        # PE warmup: ~25 back-to-back K=1 matmuls engage HAM's full clock
        # before the real instruction stream arrives
        with tc.tile_pool(name="warm_ps", bufs=1, space="PSUM") as warm_ps:
            wt = warm_ps.tile([128, 512], F32)
            for _ in range(25):
                nc.tensor.matmul(wt, lhsT=ones512[:, 0:128], rhs=ones512,
                                 start=True, stop=True)

        # x^T: two [128, 8, 2048] tiles (m-halves), DMA'd in su-major
        # 512-col slices (one DMA per su) so QKV(su) starts early
        xt_r = xt.rearrange("(c p) m -> p c m", p=128)
        xt_h = {h: singles.tile([128, KCH, S], BF16, tag=f"xt_h{h}",
                                name=f"xt_h{h}") for h in range(2)}
        for su in range(8):
            h, o = su // 4, (su % 4) * 512
            nc.sync.dma_start(out=xt_h[h][:, :, o:o + 512],
                              in_=xt_r[:, :, h * S + o:h * S + o + 512])

        def xt_cols(c, m0, m1):
            """slice of x^T chunk c for global columns [m0, m1)"""
            h = m0 // S
            assert m1 <= (h + 1) * S
            return xt_h[h][:, c, m0 - h * S:m1 - h * S]

        # ------------- shared psum pools (8 banks total) -------------
        s_ps = es.enter_context(tc.tile_pool(name="s_ps", bufs=2, space="PSUM"))
        ctx_ps = es.enter_context(tc.tile_pool(name="ctx_ps", bufs=2, space="PSUM"))
        mm_ps = es.enter_context(tc.tile_pool(name="mm_ps", bufs=2, space="PSUM"))

        pt_pool = es.enter_context(tc.tile_pool(name="pt", bufs=4))
        row_pool = es.enter_context(tc.tile_pool(name="row", bufs=2))
        rec_pool = es.enter_context(tc.tile_pool(name="rec", bufs=2))
        cs_pool = es.enter_context(tc.tile_pool(name="cs", bufs=4))
        cg_pool = es.enter_context(tc.tile_pool(name="cg", bufs=3))
        osb = es.enter_context(tc.tile_pool(name="osb", bufs=3))

        # ------------- QKV emitters -------------
        def emit_qk(su, jt, use_act=False):
            """Q^T (jt=0) or K^T (jt=1) for row-super su (512 cols)."""
            dst = qT if jt == 0 else kT
            ps = mm_ps.tile([128, 512], F32, tag="mm")
            if not use_act:   # seed bias via K=1 outer product
                nc.tensor.matmul(ps, lhsT=bias_row[:, jt * 128:(jt + 1) * 128],
                                 rhs=ones512, start=True, stop=False)
            for kc in range(KCH):
                nc.tensor.matmul(
                    ps,
                    lhsT=wqk_sb[:, kc, jt * 128:(jt + 1) * 128],
                    rhs=xt_cols(kc, su * 512, (su + 1) * 512),
                    start=(use_act and kc == 0), stop=(kc == KCH - 1),
                )
            dslice = dst[:, su * 512:(su + 1) * 512]
            if use_act:   # ACT idle in prologue: fused bias-add evacuation
                nc.scalar.add(dslice, ps, bqk_sb[:, jt:jt + 1])
            else:
                nc.vector.tensor_copy(dslice, ps)

        def emit_v(st):
            """V (row-major) for global s-tile st (128 rows)."""
            ps = mm_ps.tile([128, 512], F32, tag="mm")
            nc.tensor.matmul(ps[:, 0:128], lhsT=ones512[:, 0:128],
                             rhs=bias_row[:, 256:384], start=True, stop=False)
            for kc in range(KCH):
                nc.tensor.matmul(
                    ps[:, 0:128],
                    lhsT=xt_cols(kc, st * 128, (st + 1) * 128),
                    rhs=wv_sb[:, kc, :],
                    start=False, stop=(kc == KCH - 1),
                )
            for hl in range(HPC):
                nc.vector.tensor_copy(
                    v_sb[:, st, hl * 65:hl * 65 + 64],
                    ps[:, hl * 64:(hl + 1) * 64])

        # ------------- collective tiles: per (batch, half) -------------
        ctx_local, ctx_all = {}, {}
        for b in range(B):
            for h in range(2):
                ctx_local[(b, h)] = dram.tile(
                    [128, 1024], BF16, tag=f"ctxl{b}{h}", name=f"ctxl{b}{h}")
                ctx_all[(b, h)] = dram.tile(
                    [NCORES * 128, 1024], BF16, addr_space="Shared",
                    tag=f"ctxa{b}{h}", name=f"ctxa{b}{h}")

        # ------------- c_proj emitters (two-phase) -------------
        # phase 1 (emit_cg): issue the 8 gather DMAs on the sync queue --
        # their AllGather wait only blocks later gathers, never the PE.
        # phase 2 (emit_cpmm): the matmuls, popped >=1 q-group later so the
        # gathered data is resident when the in-order PE stream reaches them.
        cg_sets = {}

        def emit_cg(b, h):
            ca = ctx_all[(b, h)]
            # [1024, 1024] rank-major rows -> [128, 8, 1024] (p, c, m)
            src = bass.AP(tensor=ca.tensor, offset=ca.offset,
                          ap=[[1024, 128], [128 * 1024, NCORES], [1, 1024]])
            cg = cg_pool.tile([128, NCORES, 1024], BF16, tag="cg")
            nc.sync.dma_start(out=cg, in_=src)
            cg_sets[(b, h)] = cg

        def emit_cpmm(b, h, sub, last=False):
            """output cols [b*S + h*1024 + sub*512, +512), transposed."""
            cg = cg_sets[(b, h)]
            if last:
                del cg_sets[(b, h)]
            ps = mm_ps.tile([128, 512], F32, tag="mm")
            nc.tensor.matmul(ps, lhsT=bias_row[:, 384:512], rhs=ones512,
                             start=True, stop=False)
            for c in range(NCORES):
                nc.tensor.matmul(
                    ps, lhsT=wp_sb[:, c, :],
                    rhs=cg[:, c, sub * 512:(sub + 1) * 512],
                    start=False, stop=(c == NCORES - 1),
                )
            o = osb.tile([128, 512], F32, tag="o")
            nc.vector.tensor_copy(o, ps)
            col = b * S + h * 1024 + sub * 512
            nc.gpsimd.dma_start(out=outT[:, col:col + 512], in_=o)

        # ------------- attention -------------
        def emit_attn(b, fill, fill_per_kt, add_after_g=None):
            """Attention for batch b.  Per-kt pipeline: scores(kt+1) is
            emitted before AV(kt).  Pops fill-units between kt steps;
            add_after_g[g] units join the queue only after g's epilogue."""
            for g in range(G_PER_B):
                n_kt = 4 * g + 4
                cps = [ctx_ps.tile([65, 512], F32, tag="ctx", name=f"cps{_hl}")
                       for _hl in range(HPC)]
                q_sl = [qT[hl * 64:(hl + 1) * 64,
                           b * S + g * 512:b * S + (g + 1) * 512]
                        for hl in range(HPC)]
                pend_av = None
                for kt in range(n_kt):
                    qo = max(kt - 4 * g, 0) * 128  # causal trim offset
                    sp = s_ps.tile([128, 2, 512], F32, tag="s")
                    pt = pt_pool.tile([128, 2, 512], BF16, tag="pt")
                    for hl in range(HPC):
                        nc.tensor.matmul(
                            sp[:, hl, qo:512],
                            lhsT=kT[hl * 64:(hl + 1) * 64,
                                    b * S + kt * 128:b * S + (kt + 1) * 128],
                            rhs=q_sl[hl][:, qo:512],
                            start=True, stop=True,
                            tile_position=(64 * hl, 0),
                        )
                    if qo > 0:
                        nc.vector.memset(pt[:, :, 0:qo], 0.0)
                    nc.scalar.activation(pt[:, :, qo:512], sp[:, :, qo:512],
                                         EXP, scale=SCALE)
                    if kt >= 4 * g:   # diagonal block mask, both heads
                        nc.vector.tensor_mul(
                            pt[:, :, qo:qo + 128], pt[:, :, qo:qo + 128],
                            mask_sb)
                    if pend_av is not None:
                        pend_av()
                    def av(kt=kt, pt=pt):
                        for hl in range(HPC):
                            nc.tensor.matmul(
                                cps[hl],
                                lhsT=v_sb[:, b * ST_B + kt,
                                          hl * 65:hl * 65 + 65],
                                rhs=pt[:, hl, :],
                                start=(kt == 0), stop=(kt == n_kt - 1),
                            )
                    pend_av = av
                    for _ in range(fill_per_kt):
                        if fill:
                            fill.pop(0)()
                pend_av()
                # per-g normalize + ctx out: copy the sums row, broadcast it
                # across partitions with a K=1 ones matmul, fast reciprocal,
                # then scale ctx straight out of PSUM
                for hl in range(HPC):
                    row = row_pool.tile([65, 512], F32, tag="row")
                    nc.vector.tensor_copy(row[64:65, :], cps[hl][64:65, :])
                    bc_ps = mm_ps.tile([128, 512], F32, tag="mm")
                    nc.tensor.matmul(bc_ps[0:64, :], lhsT=ones_p64[64:65, :],
                                     rhs=row[64:65, :], start=True, stop=True,
                                     tile_position=(64, 0))
                    rec = rec_pool.tile([64, 512], F32, tag="rec")
                    nc.vector.reciprocal_approx_fast(rec, bc_ps[0:64, :])
                    cs = cs_pool.tile([64, 512], BF16, tag="cs")
                    nc.vector.tensor_mul(cs, cps[hl][0:64, :], rec)
                    nc.gpsimd.dma_start(
                        out=ctx_local[(b, g // 2)][hl * 64:(hl + 1) * 64,
                                                   (g % 2) * 512:
                                                   (g % 2) * 512 + 512],
                        in_=cs)
                    if dbg is not None:
                        gi = b * 8 + g * 2 + hl
                        nc.gpsimd.dma_start(out=dbg["sums"][:, gi, :],
                                            in_=row[64:65, :])
                        nc.gpsimd.dma_start(out=dbg["rec"][:, gi, :], in_=rec)
                        nc.gpsimd.dma_start(
                            out=dbg["ctx"][hl * 64:(hl + 1) * 64, b * 4 + g, :],
                            in_=cs)
                if g % 2 == 1:   # per-half-batch AllGather
                    h = g // 2
                    nc.gpsimd.collective_compute(
                        "AllGather", mybir.AluOpType.bypass, replica_groups=rg,
                        ins=[ctx_local[(b, h)].opt()],
                        outs=[ctx_all[(b, h)].opt()],
                    )
                if add_after_g and g in add_after_g:
                    fill.extend(add_after_g[g])
            return fill

        # ------------- choreography -------------
        # minimal QKV prologue for attention(b0) g0 (ACT evacuation: idle)
        emit_qk(0, 0, use_act=True)
        emit_qk(0, 1, use_act=True)
        for st in range(4):
            emit_v(st)

        # rest of QKV b0 (ordered so g deps are met), then QKV b1
        fill = []
        for su in range(1, 8):
            fill.append(lambda su=su: emit_qk(su, 0))
            fill.append(lambda su=su: emit_qk(su, 1))
            for st in range(su * 4, su * 4 + 4):
                fill.append(lambda st=st: emit_v(st))

        cg_u = lambda b, h: (lambda: emit_cg(b, h))
        mm_u = lambda b, h, sub, last=False: (lambda: emit_cpmm(b, h, sub, last))
        after0 = {2: [cg_u(0, 0)]}
        fill = emit_attn(0, fill, 1, add_after_g=after0)
        for f in fill:   # leftovers
            f()

        # attention b1, interleaving c_proj(b0) and c_proj(b1)
        fill2 = [mm_u(0, 0, 0), mm_u(0, 0, 1, True), cg_u(0, 1),
                 mm_u(0, 1, 0), mm_u(0, 1, 1, True)]
        after1 = {1: [cg_u(1, 0)],
                  2: [mm_u(1, 0, 0), mm_u(1, 0, 1, True)],
                  3: [cg_u(1, 1)]}
        fill2 = emit_attn(1, fill2, 1, add_after_g=after1)
        for f in fill2:
            f()
        emit_cpmm(1, 1, 0)
        emit_cpmm(1, 1, 1, True)

        if dbg is not None:
            nc.sync.dma_start(out=dbg["qkt"][:, 0, :], in_=qT)
            nc.sync.dma_start(out=dbg["qkt"][:, 1, :], in_=kT)
            nc.sync.dma_start(out=dbg["v"], in_=v_sb)


_CACHE = {}


def _get_compiled():
    if "nc" not in _CACHE:
        nc = bacc.Bacc("TRN2", target_bir_lowering=False, debug=False,
                       num_devices=NCORES)
        build_ir(nc)
        nc.compile()
        _CACHE["nc"] = nc
    return _CACHE["nc"]


def make_in_maps(inputs):
    x = np.asarray(inputs["hidden_states"], dtype=np.float32)   # [B,S,D]
    wa = np.asarray(inputs["c_attn_w"], dtype=np.float32)       # [D, 3D]
    ba = np.asarray(inputs["c_attn_b"], dtype=np.float32)       # [3D]
    wpr = np.asarray(inputs["c_proj_w"], dtype=np.float32)      # [D, D]
    bpr = np.asarray(inputs["c_proj_b"], dtype=np.float32)      # [D]

    bf = ml_dtypes.bfloat16
    xT = np.ascontiguousarray(x.reshape(M, D).T).astype(bf)     # [D, M]
    wq, wk, wv_full = wa[:, 0:D], wa[:, D:2 * D], wa[:, 2 * D:3 * D]
    bq, bk, bv_full = ba[0:D], ba[D:2 * D], ba[2 * D:3 * D]

    in_maps = []
    for r in range(NCORES):
        hs = slice(r * HPC * HD, (r + 1) * HPC * HD)   # this core's head dims
        bqk_r = np.concatenate([bq[hs], bk[hs]])
        in_maps.append({
            "xt": xT,
            "wqk": np.ascontiguousarray(
                np.concatenate([wq[:, hs], wk[:, hs]], axis=1)).astype(bf),
            "wv": np.ascontiguousarray(wv_full[:, hs]).astype(bf),
            "wp": np.ascontiguousarray(wpr[:, r * 128:(r + 1) * 128]).astype(bf),
            "bqk": np.ascontiguousarray(bqk_r),
            "biases": np.ascontiguousarray(np.concatenate(
                [bqk_r, bv_full[hs], bpr[r * 128:(r + 1) * 128]])).astype(bf),
        })
    return in_maps


def assemble(results):
    slices = [results[r]["outT"].T.reshape(B, S, 128) for r in range(NCORES)]
    return np.ascontiguousarray(np.concatenate(slices, axis=2).astype(np.float32))


def kernel(**inputs):
    in_maps = make_in_maps(inputs)
    nc = _get_compiled()
    res = run_bass_kernel_spmd(nc, in_maps, core_ids=list(range(NCORES)))
    return assemble(res.results)


if __name__ == "__main__":
    import reference
    inp = reference.setup_inputs()
    out = kernel(**{k: np.asarray(v) for k, v in inp.items()})
    print(out.shape, out.dtype)


# revision 34
# speedup vs baseline: 1.1084x; 1.1084x over previous
"""Trainium2 Bass kernel for ClassicAttention (B=2, S=2048, D=1024, H=16).

Sharding: tensor-parallel over heads across 8 cores (2 heads/core).
  - Host pre-transposes x to x^T [D, M] and pre-casts all matmul operands
    to bf16, so the kernel has no cast / AllGather / DMA-transpose prologue.
  - QKV projection: each core computes Q^T,K^T (d-major) and V (row-major)
    for its 2 heads over all B*S rows straight from x^T in SBUF.  Biases are
    seeded into PSUM with K=1 outer-product matmuls; evacuation is a plain
    DVE copy (tensor_scalar_add from PSUM measured pathologically slow).
  - Attention: transposed-scores formulation S^T[k,q]; both heads share one
    [128,1024] score tile so each k-tile needs a single exp ACTIVATE.  The
    softmax denominator rides row 64 of the AV accumulator via a ones
    column in V.  Per-kt software pipeline: scores(kt+1) is emitted before
    AV(kt) so the exp(kt) runs while the PE does useful work.  Per-q-group
    normalization broadcasts the sums row with a K=1 ones matmul.
  - c_proj: per-(batch, q-group) AllGather of context (8x 128KB), input
    gathers on the gpsimd DMA queue so AllGather waits never block the
    sync queue; each core computes a 128-column slice of the output,
    transposed ([j, B*S]); the host transposes back.
  - Emission order software-pipelines phases: QKV(b1) matmuls interleave
    into attention(b0)'s PE stream, c_proj units into attention(b1).
All matmuls bf16 inputs with fp32 PSUM accumulation.
"""

import numpy as np
import ml_dtypes

import concourse.bass as bass
import concourse.tile as tile
import concourse.mybir as mybir
from concourse import bacc
from concourse.bass_utils import run_bass_kernel_spmd

F32 = mybir.dt.float32
BF16 = mybir.dt.bfloat16

NCORES = 8
B, S, D = 2, 2048, 1024
H, HD = 16, 64
HPC = H // NCORES          # heads per core = 2
M = B * S                  # 4096 rows
ST_B = S // 128            # 16 s-tiles per batch
KCH = D // 128             # 8 contraction chunks
G_PER_B = S // 512         # 4 q-supers per batch
SCALE = 1.0 / (HD ** 0.5)
EXP = mybir.ActivationFunctionType.Exp
DEBUG = False


def build_ir(nc):
    # ---------------- DRAM I/O ----------------
    xt = nc.dram_tensor("xt", [D, M], BF16, kind="ExternalInput").ap()
    wqk = nc.dram_tensor("wqk", [D, 256], BF16, kind="ExternalInput").ap()
    wv = nc.dram_tensor("wv", [D, 128], BF16, kind="ExternalInput").ap()
    wp = nc.dram_tensor("wp", [D, 128], BF16, kind="ExternalInput").ap()
    bqk = nc.dram_tensor("bqk", [256], F32, kind="ExternalInput").ap()
    biases = nc.dram_tensor("biases", [512], BF16, kind="ExternalInput").ap()
    outT = nc.dram_tensor("outT", [128, M], F32, kind="ExternalOutput").ap()

    # causal mask for the diagonal 128-block: mask[k, c] = 1 if c >= k,
    # duplicated for both heads ([128, 2, 128]) so one mul covers a kt tile
    mask_np = (np.arange(128)[None, :] >= np.arange(128)[:, None])
    mask2 = np.stack([mask_np, mask_np], axis=1)
    mask_const = nc.inline_tensor(mask2.astype(ml_dtypes.bfloat16),
                                  "mask_const").ap()

    rg = [list(range(NCORES))]

    dbg = None
    if DEBUG:
        dbg = {
            "qkt": nc.dram_tensor("dbg_qkt", [128, 2, M], BF16,
                                  kind="ExternalOutput").ap(),
            "v": nc.dram_tensor("dbg_v", [128, B * ST_B, 130], BF16,
                                kind="ExternalOutput").ap(),
            "sums": nc.dram_tensor("dbg_sums", [1, 16, 512], F32,
                                   kind="ExternalOutput").ap(),
            "rec": nc.dram_tensor("dbg_rec", [64, 16, 512], F32,
                                  kind="ExternalOutput").ap(),
            "ctx": nc.dram_tensor("dbg_ctx", [128, 8, 512], BF16,
                                  kind="ExternalOutput").ap(),
        }

    with tile.TileContext(nc) as tc:
        _emit(nc, tc, xt, wqk, wv, wp, bqk, biases, outT, mask_const, rg, dbg)
    return nc


def _emit(nc, tc, xt, wqk, wv, wp, bqk, biases, outT, mask_const, rg, dbg=None):
    import contextlib
    es = contextlib.ExitStack()
    with es:
        singles = es.enter_context(tc.tile_pool(name="singles", bufs=1))
        dram = es.enter_context(tc.tile_pool(name="dram", bufs=1, space="DRAM"))

        # ------------- persistent SBUF -------------
        qT = singles.tile([128, M], BF16, tag="qT")
        kT = singles.tile([128, M], BF16, tag="kT")
        v_sb = singles.tile([128, B * ST_B, 130], BF16, tag="v_sb")
        mask_sb = singles.tile([128, 2, 128], BF16, tag="mask_sb")
        nc.sync.dma_start(out=mask_sb, in_=mask_const)
        nc.vector.memset(v_sb, 1.0)                # ones columns pre-set

        # weights (already bf16 from host)
        wqk_sb = singles.tile([128, KCH, 256], BF16, tag="wqk_sb")
        wv_sb = singles.tile([128, KCH, 128], BF16, tag="wv_sb")
        wp_sb = singles.tile([128, KCH, 128], BF16, tag="wp_sb")
        bqk_sb = singles.tile([128, 2], F32, tag="bqk_sb")
        # bias_row: [bqk(256) | bv(128) | bp(128)] bf16, for K=1 seed matmuls
        bias_row = singles.tile([1, 512], BF16, tag="bias_row")
        ones512 = singles.tile([1, 512], BF16, tag="ones512")
        ones_p64 = singles.tile([65, 64], F32, tag="ones_p64")
        nc.vector.memset(ones512, 1.0)
        nc.vector.memset(ones_p64, 1.0)
        nc.sync.dma_start(out=wqk_sb, in_=wqk.rearrange("(c p) j -> p c j", p=128))
        nc.sync.dma_start(out=wv_sb, in_=wv.rearrange("(c p) j -> p c j", p=128))
        nc.sync.dma_start(out=wp_sb, in_=wp.rearrange("(c p) j -> p c j", p=128))
        nc.sync.dma_start(out=bqk_sb, in_=bqk.rearrange("(t p) -> p t", p=128))
        nc.sync.dma_start(out=bias_row, in_=biases.rearrange("(a j) -> a j", a=1))

        # PE warmup: ~25 back-to-back K=1 matmuls engage HAM's full clock
        # before the real instruction stream arrives
        with tc.tile_pool(name="warm_ps", bufs=1, space="PSUM") as warm_ps:
            wt = warm_ps.tile([128, 512], F32)
            for _ in range(25):
                nc.tensor.matmul(wt, lhsT=ones512[:, 0:128], rhs=ones512,
                                 start=True, stop=True)

        # x^T: two [128, 8, 2048] tiles (m-halves), DMA'd in su-major
        # 512-col slices (one DMA per su) so QKV(su) starts early
        xt_r = xt.rearrange("(c p) m -> p c m", p=128)
        xt_h = {h: singles.tile([128, KCH, S], BF16, tag=f"xt_h{h}",
                                name=f"xt_h{h}") for h in range(2)}
        for su in range(8):
            h, o = su // 4, (su % 4) * 512
            nc.sync.dma_start(out=xt_h[h][:, :, o:o + 512],
                              in_=xt_r[:, :, h * S + o:h * S + o + 512])

        def xt_cols(c, m0, m1):
            """slice of x^T chunk c for global columns [m0, m1)"""
            h = m0 // S
            assert m1 <= (h + 1) * S
            return xt_h[h][:, c, m0 - h * S:m1 - h * S]

        # ------------- shared psum pools (8 banks total) -------------
        s_ps = es.enter_context(tc.tile_pool(name="s_ps", bufs=2, space="PSUM"))
        ctx_ps = es.enter_context(tc.tile_pool(name="ctx_ps", bufs=2, space="PSUM"))
        mm_ps = es.enter_context(tc.tile_pool(name="mm_ps", bufs=2, space="PSUM"))

        pt_pool = es.enter_context(tc.tile_pool(name="pt", bufs=4))
        row_pool = es.enter_context(tc.tile_pool(name="row", bufs=2))
        rec_pool = es.enter_context(tc.tile_pool(name="rec", bufs=2))
        cs_pool = es.enter_context(tc.tile_pool(name="cs", bufs=4))
        cg_pool = es.enter_context(tc.tile_pool(name="cg", bufs=3))
        osb = es.enter_context(tc.tile_pool(name="osb", bufs=3))

        # ------------- QKV emitters -------------
        def emit_qk(su, jt, use_act=False):
            """Q^T (jt=0) or K^T (jt=1) for row-super su (512 cols)."""
            dst = qT if jt == 0 else kT
            ps = mm_ps.tile([128, 512], F32, tag="mm")
            if not use_act:   # seed bias via K=1 outer product
                nc.tensor.matmul(ps, lhsT=bias_row[:, jt * 128:(jt + 1) * 128],
                                 rhs=ones512, start=True, stop=False)
            for kc in range(KCH):
                nc.tensor.matmul(
                    ps,
                    lhsT=wqk_sb[:, kc, jt * 128:(jt + 1) * 128],
                    rhs=xt_cols(kc, su * 512, (su + 1) * 512),
                    start=(use_act and kc == 0), stop=(kc == KCH - 1),
                )
            dslice = dst[:, su * 512:(su + 1) * 512]
            if use_act:   # ACT idle in prologue: fused bias-add evacuation
                nc.scalar.add(dslice, ps, bqk_sb[:, jt:jt + 1])
            else:
                nc.vector.tensor_copy(dslice, ps)

        def emit_v(st):
            """V (row-major) for global s-tile st (128 rows)."""
            ps = mm_ps.tile([128, 512], F32, tag="mm")
            nc.tensor.matmul(ps[:, 0:128], lhsT=ones512[:, 0:128],
                             rhs=bias_row[:, 256:384], start=True, stop=False)
            for kc in range(KCH):
                nc.tensor.matmul(
                    ps[:, 0:128],
                    lhsT=xt_cols(kc, st * 128, (st + 1) * 128),
                    rhs=wv_sb[:, kc, :],
                    start=False, stop=(kc == KCH - 1),
                )
            for hl in range(HPC):
                nc.vector.tensor_copy(
                    v_sb[:, st, hl * 65:hl * 65 + 64],
                    ps[:, hl * 64:(hl + 1) * 64])

        # ------------- collective tiles: per (batch, half) -------------
        ctx_local, ctx_all = {}, {}
        for b in range(B):
            for h in range(2):
                ctx_local[(b, h)] = dram.tile(
                    [128, 1024], BF16, tag=f"ctxl{b}{h}", name=f"ctxl{b}{h}")
                ctx_all[(b, h)] = dram.tile(
                    [NCORES * 128, 1024], BF16, addr_space="Shared",
                    tag=f"ctxa{b}{h}", name=f"ctxa{b}{h}")

        # ------------- c_proj emitters (two-phase) -------------
        # phase 1 (emit_cg): issue the 8 gather DMAs on the sync queue --
        # their AllGather wait only blocks later gathers, never the PE.
        # phase 2 (emit_cpmm): the matmuls, popped >=1 q-group later so the
        # gathered data is resident when the in-order PE stream reaches them.
        cg_sets = {}

        def emit_cg(b, h):
            ca = ctx_all[(b, h)]
            # [1024, 1024] rank-major rows -> [128, 8, 1024] (p, c, m)
            src = bass.AP(tensor=ca.tensor, offset=ca.offset,
                          ap=[[1024, 128], [128 * 1024, NCORES], [1, 1024]])
            cg = cg_pool.tile([128, NCORES, 1024], BF16, tag="cg")
            nc.sync.dma_start(out=cg, in_=src)
            cg_sets[(b, h)] = cg

        def emit_cpmm(b, h, sub, last=False):
            """output cols [b*S + h*1024 + sub*512, +512), transposed."""
            cg = cg_sets[(b, h)]
            if last:
                del cg_sets[(b, h)]
            ps = mm_ps.tile([128, 512], F32, tag="mm")
            nc.tensor.matmul(ps, lhsT=bias_row[:, 384:512], rhs=ones512,
                             start=True, stop=False)
            for c in range(NCORES):
                nc.tensor.matmul(
                    ps, lhsT=wp_sb[:, c, :],
                    rhs=cg[:, c, sub * 512:(sub + 1) * 512],
                    start=False, stop=(c == NCORES - 1),
                )
            o = osb.tile([128, 512], F32, tag="o")
            nc.vector.tensor_copy(o, ps)
            col = b * S + h * 1024 + sub * 512
            nc.scalar.dma_start(out=outT[:, col:col + 512], in_=o)

        # ------------- attention -------------
        def emit_attn(b, fill, fill_per_kt, add_after_g=None):
            """Attention for batch b.  Per-kt pipeline: scores(kt+1) is
            emitted before AV(kt).  Pops fill-units between kt steps;
            add_after_g[g] units join the queue only after g's epilogue."""
            for g in range(G_PER_B):
                n_kt = 4 * g + 4
                cps = [ctx_ps.tile([65, 512], F32, tag="ctx", name=f"cps{_hl}")
                       for _hl in range(HPC)]
                q_sl = [qT[hl * 64:(hl + 1) * 64,
                           b * S + g * 512:b * S + (g + 1) * 512]
                        for hl in range(HPC)]
                pend_av = None
                for kt in range(n_kt):
                    qo = max(kt - 4 * g, 0) * 128  # causal trim offset
                    sp = s_ps.tile([128, 2, 512], F32, tag="s")
                    pt = pt_pool.tile([128, 2, 512], BF16, tag="pt")
                    for hl in range(HPC):
                        nc.tensor.matmul(
                            sp[:, hl, qo:512],
                            lhsT=kT[hl * 64:(hl + 1) * 64,
                                    b * S + kt * 128:b * S + (kt + 1) * 128],
                            rhs=q_sl[hl][:, qo:512],
                            start=True, stop=True,
                            tile_position=(64 * hl, 0),
                        )
                    if qo > 0:
                        nc.vector.memset(pt[:, :, 0:qo], 0.0)
                    nc.scalar.activation(pt[:, :, qo:512], sp[:, :, qo:512],
                                         EXP, scale=SCALE)
                    if kt >= 4 * g:   # diagonal block mask, both heads
                        nc.vector.tensor_mul(
                            pt[:, :, qo:qo + 128], pt[:, :, qo:qo + 128],
                            mask_sb)
                    if pend_av is not None:
                        pend_av()
                    def av(kt=kt, pt=pt):
                        for hl in range(HPC):
                            nc.tensor.matmul(
                                cps[hl],
                                lhsT=v_sb[:, b * ST_B + kt,
                                          hl * 65:hl * 65 + 65],
                                rhs=pt[:, hl, :],
                                start=(kt == 0), stop=(kt == n_kt - 1),
                            )
                    pend_av = av
                    for _ in range(fill_per_kt):
                        if fill:
                            fill.pop(0)()
                pend_av()
                # per-g normalize + ctx out: copy the sums row, broadcast it
                # across partitions with a K=1 ones matmul, fast reciprocal,
                # then scale ctx straight out of PSUM
                for hl in range(HPC):
                    row = row_pool.tile([65, 512], F32, tag="row")
                    nc.vector.tensor_copy(row[64:65, :], cps[hl][64:65, :])
                    bc_ps = mm_ps.tile([128, 512], F32, tag="mm")
                    nc.tensor.matmul(bc_ps[0:64, :], lhsT=ones_p64[64:65, :],
                                     rhs=row[64:65, :], start=True, stop=True,
                                     tile_position=(64, 0))
                    rec = rec_pool.tile([64, 512], F32, tag="rec")
                    nc.vector.reciprocal_approx_fast(rec, bc_ps[0:64, :])
                    cs = cs_pool.tile([64, 512], BF16, tag="cs")
                    nc.vector.tensor_mul(cs, cps[hl][0:64, :], rec)
                    nc.gpsimd.dma_start(
                        out=ctx_local[(b, g // 2)][hl * 64:(hl + 1) * 64,
                                                   (g % 2) * 512:
                                                   (g % 2) * 512 + 512],
                        in_=cs)
                    if dbg is not None:
                        gi = b * 8 + g * 2 + hl
                        nc.gpsimd.dma_start(out=dbg["sums"][:, gi, :],
                                            in_=row[64:65, :])
                        nc.gpsimd.dma_start(out=dbg["rec"][:, gi, :], in_=rec)
                        nc.gpsimd.dma_start(
                            out=dbg["ctx"][hl * 64:(hl + 1) * 64, b * 4 + g, :],
                            in_=cs)
                if g % 2 == 1:   # per-half-batch AllGather
                    h = g // 2
                    nc.gpsimd.collective_compute(
                        "AllGather", mybir.AluOpType.bypass, replica_groups=rg,
                        ins=[ctx_local[(b, h)].opt()],
                        outs=[ctx_all[(b, h)].opt()],
                    )
                if add_after_g and g in add_after_g:
                    fill.extend(add_after_g[g])
            return fill

        # ------------- choreography -------------
        # minimal QKV prologue for attention(b0) g0 (ACT evacuation: idle)
        emit_qk(0, 0, use_act=True)
        emit_qk(0, 1, use_act=True)
        for st in range(4):
            emit_v(st)

        # rest of QKV b0 (ordered so g deps are met), then QKV b1
        fill = []
        for su in range(1, 8):
            fill.append(lambda su=su: emit_qk(su, 0))
            fill.append(lambda su=su: emit_qk(su, 1))
            for st in range(su * 4, su * 4 + 4):
                fill.append(lambda st=st: emit_v(st))

        cg_u = lambda b, h: (lambda: emit_cg(b, h))
        mm_u = lambda b, h, sub, last=False: (lambda: emit_cpmm(b, h, sub, last))
        after0 = {2: [cg_u(0, 0)]}
        fill = emit_attn(0, fill, 1, add_after_g=after0)
        for f in fill:   # leftovers
            f()

        # attention b1, interleaving c_proj(b0) and c_proj(b1)
        fill2 = [mm_u(0, 0, 0), mm_u(0, 0, 1, True), cg_u(0, 1),
                 mm_u(0, 1, 0), mm_u(0, 1, 1, True)]
        after1 = {1: [cg_u(1, 0)],
                  2: [mm_u(1, 0, 0), mm_u(1, 0, 1, True)],
                  3: [cg_u(1, 1)]}
        fill2 = emit_attn(1, fill2, 1, add_after_g=after1)
        for f in fill2:
            f()
        emit_cpmm(1, 1, 0)
        emit_cpmm(1, 1, 1, True)

        if dbg is not None:
            nc.sync.dma_start(out=dbg["qkt"][:, 0, :], in_=qT)
            nc.sync.dma_start(out=dbg["qkt"][:, 1, :], in_=kT)
            nc.sync.dma_start(out=dbg["v"], in_=v_sb)


_CACHE = {}


def _get_compiled():
    if "nc" not in _CACHE:
        nc = bacc.Bacc("TRN2", target_bir_lowering=False, debug=False,
                       num_devices=NCORES)
        build_ir(nc)
        nc.compile()
        _CACHE["nc"] = nc
    return _CACHE["nc"]


def make_in_maps(inputs):
    x = np.asarray(inputs["hidden_states"], dtype=np.float32)   # [B,S,D]
    wa = np.asarray(inputs["c_attn_w"], dtype=np.float32)       # [D, 3D]
    ba = np.asarray(inputs["c_attn_b"], dtype=np.float32)       # [3D]
    wpr = np.asarray(inputs["c_proj_w"], dtype=np.float32)      # [D, D]
    bpr = np.asarray(inputs["c_proj_b"], dtype=np.float32)      # [D]

    bf = ml_dtypes.bfloat16
    xT = np.ascontiguousarray(x.reshape(M, D).T).astype(bf)     # [D, M]
    wq, wk, wv_full = wa[:, 0:D], wa[:, D:2 * D], wa[:, 2 * D:3 * D]
    bq, bk, bv_full = ba[0:D], ba[D:2 * D], ba[2 * D:3 * D]

    in_maps = []
    for r in range(NCORES):
        hs = slice(r * HPC * HD, (r + 1) * HPC * HD)   # this core's head dims
        bqk_r = np.concatenate([bq[hs], bk[hs]])
        in_maps.append({
            "xt": xT,
            "wqk": np.ascontiguousarray(
                np.concatenate([wq[:, hs], wk[:, hs]], axis=1)).astype(bf),
            "wv": np.ascontiguousarray(wv_full[:, hs]).astype(bf),
            "wp": np.ascontiguousarray(wpr[:, r * 128:(r + 1) * 128]).astype(bf),
            "bqk": np.ascontiguousarray(bqk_r),
            "biases": np.ascontiguousarray(np.concatenate(
                [bqk_r, bv_full[hs], bpr[r * 128:(r + 1) * 128]])).astype(bf),
        })
    return in_maps


def assemble(results):
    slices = [results[r]["outT"].T.reshape(B, S, 128) for r in range(NCORES)]
    return np.ascontiguousarray(np.concatenate(slices, axis=2).astype(np.float32))


def kernel(**inputs):
    in_maps = make_in_maps(inputs)
    nc = _get_compiled()
    res = run_bass_kernel_spmd(nc, in_maps, core_ids=list(range(NCORES)))
    return assemble(res.results)


if __name__ == "__main__":
    import reference
    inp = reference.setup_inputs()
    out = kernel(**{k: np.asarray(v) for k, v in inp.items()})
    print(out.shape, out.dtype)


# revision 39
# speedup vs baseline: 1.1674x; 1.0533x over previous
"""Trainium2 Bass kernel for ClassicAttention (B=2, S=2048, D=1024, H=16).

Sharding: tensor-parallel over heads across 8 cores (2 heads/core).
  - Host pre-transposes x to x^T [D, M] and pre-casts all matmul operands
    to bf16, so the kernel has no cast / AllGather / DMA-transpose prologue.
  - QKV projection: each core computes Q^T,K^T (d-major) and V (row-major)
    for its 2 heads over all B*S rows straight from x^T in SBUF.  Biases are
    seeded into PSUM with K=1 outer-product matmuls; evacuation is a plain
    DVE copy (tensor_scalar_add from PSUM measured pathologically slow).
  - Attention: transposed-scores formulation S^T[k,q]; both heads share one
    [128,1024] score tile so each k-tile needs a single exp ACTIVATE.  The
    softmax denominator rides row 64 of the AV accumulator via a ones
    column in V.  Per-kt software pipeline: scores(kt+1) is emitted before
    AV(kt) so the exp(kt) runs while the PE does useful work.  Per-q-group
    normalization broadcasts the sums row with a K=1 ones matmul.
  - c_proj: per-(batch, q-group) AllGather of context (8x 128KB), input
    gathers on the gpsimd DMA queue so AllGather waits never block the
    sync queue; each core computes a 128-column slice of the output,
    transposed ([j, B*S]); the host transposes back.
  - Emission order software-pipelines phases: QKV(b1) matmuls interleave
    into attention(b0)'s PE stream, c_proj units into attention(b1).
All matmuls bf16 inputs with fp32 PSUM accumulation.
"""

import numpy as np
import ml_dtypes

import concourse.bass as bass
import concourse.tile as tile
import concourse.mybir as mybir
from concourse import bacc
from concourse.bass_utils import run_bass_kernel_spmd

F32 = mybir.dt.float32
BF16 = mybir.dt.bfloat16

NCORES = 8
B, S, D = 2, 2048, 1024
H, HD = 16, 64
HPC = H // NCORES          # heads per core = 2
M = B * S                  # 4096 rows
ST_B = S // 128            # 16 s-tiles per batch
KCH = D // 128             # 8 contraction chunks
G_PER_B = S // 512         # 4 q-supers per batch
SCALE = 1.0 / (HD ** 0.5)
EXP = mybir.ActivationFunctionType.Exp
DEBUG = False


def build_ir(nc):
    # ---------------- DRAM I/O ----------------
    xt = nc.dram_tensor("xt", [D, M], BF16, kind="ExternalInput").ap()
    wqk = nc.dram_tensor("wqk", [D, 256], BF16, kind="ExternalInput").ap()
    wv = nc.dram_tensor("wv", [D, 128], BF16, kind="ExternalInput").ap()
    wp = nc.dram_tensor("wp", [D, 128], BF16, kind="ExternalInput").ap()
    bqk = nc.dram_tensor("bqk", [256], F32, kind="ExternalInput").ap()
    biases = nc.dram_tensor("biases", [512], BF16, kind="ExternalInput").ap()
    outT = nc.dram_tensor("outT", [128, M], F32, kind="ExternalOutput").ap()

    # causal mask for the diagonal 128-block: mask[k, c] = 1 if c >= k,
    # duplicated for both heads ([128, 2, 128]) so one mul covers a kt tile
    mask_np = (np.arange(128)[None, :] >= np.arange(128)[:, None])
    mask2 = np.stack([mask_np, mask_np], axis=1)
    mask_const = nc.inline_tensor(mask2.astype(ml_dtypes.bfloat16),
                                  "mask_const").ap()

    rg = [list(range(NCORES))]

    dbg = None
    if DEBUG:
        dbg = {
            "qkt": nc.dram_tensor("dbg_qkt", [128, 2, M], BF16,
                                  kind="ExternalOutput").ap(),
            "v": nc.dram_tensor("dbg_v", [128, B * ST_B, 130], BF16,
                                kind="ExternalOutput").ap(),
            "sums": nc.dram_tensor("dbg_sums", [1, 16, 512], F32,
                                   kind="ExternalOutput").ap(),
            "rec": nc.dram_tensor("dbg_rec", [64, 16, 512], F32,
                                  kind="ExternalOutput").ap(),
            "ctx": nc.dram_tensor("dbg_ctx", [128, 8, 512], BF16,
                                  kind="ExternalOutput").ap(),
        }

    with tile.TileContext(nc) as tc:
        _emit(nc, tc, xt, wqk, wv, wp, bqk, biases, outT, mask_const, rg, dbg)
    return nc


def _emit(nc, tc, xt, wqk, wv, wp, bqk, biases, outT, mask_const, rg, dbg=None):
    import contextlib
    es = contextlib.ExitStack()
    with es:
        singles = es.enter_context(tc.tile_pool(name="singles", bufs=1))
        dram = es.enter_context(tc.tile_pool(name="dram", bufs=1, space="DRAM"))

        # ------------- persistent SBUF -------------
        qT = singles.tile([128, M], BF16, tag="qT")
        kT = singles.tile([128, M], BF16, tag="kT")
        v_sb = singles.tile([128, B * ST_B, 130], BF16, tag="v_sb")
        mask_sb = singles.tile([128, 2, 128], BF16, tag="mask_sb")
        nc.sync.dma_start(out=mask_sb, in_=mask_const)
        nc.vector.memset(v_sb, 1.0)                # ones columns pre-set

        # weights (already bf16 from host)
        wqk_sb = singles.tile([128, KCH, 256], BF16, tag="wqk_sb")
        wv_sb = singles.tile([128, KCH, 128], BF16, tag="wv_sb")
        wp_sb = singles.tile([128, KCH, 128], BF16, tag="wp_sb")
        bqk_sb = singles.tile([128, 2], F32, tag="bqk_sb")
        # bias_row: [bqk(256) | bv(128) | bp(128)] bf16, for K=1 seed matmuls
        bias_row = singles.tile([1, 512], BF16, tag="bias_row")
        ones512 = singles.tile([1, 512], BF16, tag="ones512")
        ones_p64 = singles.tile([65, 64], F32, tag="ones_p64")
        nc.vector.memset(ones512, 1.0)
        nc.vector.memset(ones_p64, 1.0)
        nc.sync.dma_start(out=wqk_sb, in_=wqk.rearrange("(c p) j -> p c j", p=128))
        nc.sync.dma_start(out=wv_sb, in_=wv.rearrange("(c p) j -> p c j", p=128))
        nc.sync.dma_start(out=wp_sb, in_=wp.rearrange("(c p) j -> p c j", p=128))
        nc.sync.dma_start(out=bqk_sb, in_=bqk.rearrange("(t p) -> p t", p=128))
        nc.sync.dma_start(out=bias_row, in_=biases.rearrange("(a j) -> a j", a=1))

        # PE warmup: ~25 back-to-back K=1 matmuls engage HAM's full clock
        # before the real instruction stream arrives
        with tc.tile_pool(name="warm_ps", bufs=1, space="PSUM") as warm_ps:
            wt = warm_ps.tile([128, 512], F32)
            for _ in range(15):
                nc.tensor.matmul(wt, lhsT=ones512[:, 0:128], rhs=ones512,
                                 start=True, stop=True)

        # x^T: two [128, 8, 2048] tiles (m-halves), DMA'd in su-major
        # 512-col slices (one DMA per su) so QKV(su) starts early
        xt_r = xt.rearrange("(c p) m -> p c m", p=128)
        xt_h = {h: singles.tile([128, KCH, S], BF16, tag=f"xt_h{h}",
                                name=f"xt_h{h}") for h in range(2)}
        for su in range(8):
            h, o = su // 4, (su % 4) * 512
            nc.sync.dma_start(out=xt_h[h][:, :, o:o + 512],
                              in_=xt_r[:, :, h * S + o:h * S + o + 512])

        def xt_cols(c, m0, m1):
            """slice of x^T chunk c for global columns [m0, m1)"""
            h = m0 // S
            assert m1 <= (h + 1) * S
            return xt_h[h][:, c, m0 - h * S:m1 - h * S]

        # ------------- shared psum pools (8 banks total) -------------
        s_ps = es.enter_context(tc.tile_pool(name="s_ps", bufs=2, space="PSUM"))
        ctx_ps = es.enter_context(tc.tile_pool(name="ctx_ps", bufs=2, space="PSUM"))
        mm_ps = es.enter_context(tc.tile_pool(name="mm_ps", bufs=2, space="PSUM"))

        pt_pool = es.enter_context(tc.tile_pool(name="pt", bufs=4))
        row_pool = es.enter_context(tc.tile_pool(name="row", bufs=2))
        rec_pool = es.enter_context(tc.tile_pool(name="rec", bufs=2))
        cs_pool = es.enter_context(tc.tile_pool(name="cs", bufs=4))
        cg_pool = es.enter_context(tc.tile_pool(name="cg", bufs=3))
        osb = es.enter_context(tc.tile_pool(name="osb", bufs=3))

        # ------------- QKV emitters -------------
        def emit_qk(su, jt, use_act=False):
            """Q^T (jt=0) or K^T (jt=1) for row-super su (512 cols)."""
            dst = qT if jt == 0 else kT
            ps = mm_ps.tile([128, 512], F32, tag="mm")
            if not use_act:   # seed bias via K=1 outer product
                nc.tensor.matmul(ps, lhsT=bias_row[:, jt * 128:(jt + 1) * 128],
                                 rhs=ones512, start=True, stop=False)
            for kc in range(KCH):
                nc.tensor.matmul(
                    ps,
                    lhsT=wqk_sb[:, kc, jt * 128:(jt + 1) * 128],
                    rhs=xt_cols(kc, su * 512, (su + 1) * 512),
                    start=(use_act and kc == 0), stop=(kc == KCH - 1),
                )
            dslice = dst[:, su * 512:(su + 1) * 512]
            if use_act:   # ACT idle in prologue: fused bias-add evacuation
                nc.scalar.add(dslice, ps, bqk_sb[:, jt:jt + 1])
            else:
                nc.vector.tensor_copy(dslice, ps)

        def emit_v(st):
            """V (row-major) for global s-tile st (128 rows)."""
            ps = mm_ps.tile([128, 512], F32, tag="mm")
            nc.tensor.matmul(ps[:, 0:128], lhsT=ones512[:, 0:128],
                             rhs=bias_row[:, 256:384], start=True, stop=False)
            for kc in range(KCH):
                nc.tensor.matmul(
                    ps[:, 0:128],
                    lhsT=xt_cols(kc, st * 128, (st + 1) * 128),
                    rhs=wv_sb[:, kc, :],
                    start=False, stop=(kc == KCH - 1),
                )
            for hl in range(HPC):
                nc.vector.tensor_copy(
                    v_sb[:, st, hl * 65:hl * 65 + 64],
                    ps[:, hl * 64:(hl + 1) * 64])

        # ------------- collective tiles: per (batch, q-group) -------------
        # 8 small AllGathers rather than 4 big ones: each is an inter-core
        # sync point, and frequent syncs keep the cores' pacing drift (and
        # thus per-AG wait time) small
        ctx_local, ctx_all = {}, {}
        for b in range(B):
            for g in range(G_PER_B):
                ctx_local[(b, g)] = dram.tile(
                    [128, 512], BF16, tag=f"ctxl{b}{g}", name=f"ctxl{b}{g}")
                ctx_all[(b, g)] = dram.tile(
                    [NCORES * 128, 512], BF16, addr_space="Shared",
                    tag=f"ctxa{b}{g}", name=f"ctxa{b}{g}")

        # ------------- c_proj emitters (two-phase) -------------
        # phase 1 (emit_cg): issue the 8 gather DMAs on the sync queue --
        # their AllGather wait only blocks later gathers, never the PE.
        # phase 2 (emit_cpmm): the matmuls, popped >=1 q-group later so the
        # gathered data is resident when the in-order PE stream reaches them.
        cg_sets = {}

        def emit_cg(b, g):
            ca = ctx_all[(b, g)]
            # [1024, 512] rank-major rows -> [128, 8, 512] (p, c, m)
            src = bass.AP(tensor=ca.tensor, offset=ca.offset,
                          ap=[[512, 128], [128 * 512, NCORES], [1, 512]])
            cg = cg_pool.tile([128, NCORES, 512], BF16, tag="cg")
            nc.sync.dma_start(out=cg, in_=src)
            cg_sets[(b, g)] = cg

        def emit_cpmm(b, g):
            """output cols [b*S + g*512, +512), transposed [j, m]."""
            cg = cg_sets.pop((b, g))
            ps = mm_ps.tile([128, 512], F32, tag="mm")
            nc.tensor.matmul(ps, lhsT=bias_row[:, 384:512], rhs=ones512,
                             start=True, stop=False)
            for c in range(NCORES):
                nc.tensor.matmul(
                    ps, lhsT=wp_sb[:, c, :], rhs=cg[:, c, :],
                    start=False, stop=(c == NCORES - 1),
                )
            o = osb.tile([128, 512], F32, tag="o")
            nc.vector.tensor_copy(o, ps)
            col = b * S + g * 512
            nc.scalar.dma_start(out=outT[:, col:col + 512], in_=o)

        # ------------- attention -------------
        def emit_attn(b, fill, fill_per_kt, add_after_g=None):
            """Attention for batch b.  Per-kt pipeline: scores(kt+1) is
            emitted before AV(kt).  Pops fill-units between kt steps;
            add_after_g[g] units join the queue only after g's epilogue."""
            for g in range(G_PER_B):
                n_kt = 4 * g + 4
                cps = [ctx_ps.tile([65, 512], F32, tag="ctx", name=f"cps{_hl}")
                       for _hl in range(HPC)]
                q_sl = [qT[hl * 64:(hl + 1) * 64,
                           b * S + g * 512:b * S + (g + 1) * 512]
                        for hl in range(HPC)]
                pend_av = None
                for kt in range(n_kt):
                    qo = max(kt - 4 * g, 0) * 128  # causal trim offset
                    sp = s_ps.tile([128, 2, 512], F32, tag="s")
                    pt = pt_pool.tile([128, 2, 512], BF16, tag="pt")
                    for hl in range(HPC):
                        nc.tensor.matmul(
                            sp[:, hl, qo:512],
                            lhsT=kT[hl * 64:(hl + 1) * 64,
                                    b * S + kt * 128:b * S + (kt + 1) * 128],
                            rhs=q_sl[hl][:, qo:512],
                            start=True, stop=True,
                            tile_position=(64 * hl, 0),
                        )
                    if qo > 0:
                        nc.vector.memset(pt[:, :, 0:qo], 0.0)
                    nc.scalar.activation(pt[:, :, qo:512], sp[:, :, qo:512],
                                         EXP, scale=SCALE)
                    if kt >= 4 * g:   # diagonal block mask, both heads
                        nc.vector.tensor_mul(
                            pt[:, :, qo:qo + 128], pt[:, :, qo:qo + 128],
                            mask_sb)
                    if pend_av is not None:
                        pend_av()
                    def av(kt=kt, pt=pt):
                        for hl in range(HPC):
                            nc.tensor.matmul(
                                cps[hl],
                                lhsT=v_sb[:, b * ST_B + kt,
                                          hl * 65:hl * 65 + 65],
                                rhs=pt[:, hl, :],
                                start=(kt == 0), stop=(kt == n_kt - 1),
                            )
                    pend_av = av
                    for _ in range(fill_per_kt):
                        if fill:
                            fill.pop(0)()
                pend_av()
                # per-g normalize + ctx out: copy the sums row, broadcast it
                # across partitions with a K=1 ones matmul, fast reciprocal,
                # then scale ctx straight out of PSUM
                for hl in range(HPC):
                    row = row_pool.tile([65, 512], F32, tag="row")
                    nc.vector.tensor_copy(row[64:65, :], cps[hl][64:65, :])
                    bc_ps = mm_ps.tile([128, 512], F32, tag="mm")
                    nc.tensor.matmul(bc_ps[0:64, :], lhsT=ones_p64[64:65, :],
                                     rhs=row[64:65, :], start=True, stop=True,
                                     tile_position=(64, 0))
                    rec = rec_pool.tile([64, 512], F32, tag="rec")
                    nc.vector.reciprocal_approx_fast(rec, bc_ps[0:64, :])
                    cs = cs_pool.tile([64, 512], BF16, tag="cs")
                    nc.vector.tensor_mul(cs, cps[hl][0:64, :], rec)
                    nc.gpsimd.dma_start(
                        out=ctx_local[(b, g)][hl * 64:(hl + 1) * 64, :],
                        in_=cs)
                    if dbg is not None:
                        gi = b * 8 + g * 2 + hl
                        nc.gpsimd.dma_start(out=dbg["sums"][:, gi, :],
                                            in_=row[64:65, :])
                        nc.gpsimd.dma_start(out=dbg["rec"][:, gi, :], in_=rec)
                        nc.gpsimd.dma_start(
                            out=dbg["ctx"][hl * 64:(hl + 1) * 64, b * 4 + g, :],
                            in_=cs)
                nc.gpsimd.collective_compute(
                    "AllGather", mybir.AluOpType.bypass, replica_groups=rg,
                    ins=[ctx_local[(b, g)].opt()],
                    outs=[ctx_all[(b, g)].opt()],
                )
                if add_after_g and g in add_after_g:
                    fill.extend(add_after_g[g])
            return fill

        # ------------- choreography -------------
        # minimal QKV prologue for attention(b0) g0 (ACT evacuation: idle)
        emit_qk(0, 0, use_act=True)
        emit_qk(0, 1, use_act=True)
        for st in range(4):
            emit_v(st)

        # rest of QKV b0 (ordered so g deps are met), then QKV b1
        fill = []
        for su in range(1, 8):
            fill.append(lambda su=su: emit_qk(su, 0))
            fill.append(lambda su=su: emit_qk(su, 1))
            for st in range(su * 4, su * 4 + 4):
                fill.append(lambda st=st: emit_v(st))

        cg_u = lambda b, g: (lambda: emit_cg(b, g))
        mm_u = lambda b, g: (lambda: emit_cpmm(b, g))
        after0 = {1: [cg_u(0, 0)], 2: [cg_u(0, 1)], 3: [cg_u(0, 2)]}
        fill = emit_attn(0, fill, 1, add_after_g=after0)
        for f in fill:   # leftovers
            f()

        # attention b1, interleaving c_proj(b0) and c_proj(b1)
        fill2 = [mm_u(0, 0), cg_u(0, 3), mm_u(0, 1)]
        after1 = {0: [mm_u(0, 2), cg_u(1, 0)], 1: [mm_u(0, 3), cg_u(1, 1)],
                  2: [mm_u(1, 0), cg_u(1, 2)], 3: [mm_u(1, 1), cg_u(1, 3)]}
        fill2 = emit_attn(1, fill2, 1, add_after_g=after1)
        for f in fill2:
            f()
        emit_cpmm(1, 2)
        emit_cpmm(1, 3)

        if dbg is not None:
            nc.sync.dma_start(out=dbg["qkt"][:, 0, :], in_=qT)
            nc.sync.dma_start(out=dbg["qkt"][:, 1, :], in_=kT)
            nc.sync.dma_start(out=dbg["v"], in_=v_sb)


_CACHE = {}


def _get_compiled():
    if "nc" not in _CACHE:
        nc = bacc.Bacc("TRN2", target_bir_lowering=False, debug=False,
                       num_devices=NCORES)
        build_ir(nc)
        nc.compile()
        _CACHE["nc"] = nc
    return _CACHE["nc"]


def make_in_maps(inputs):
    x = np.asarray(inputs["hidden_states"], dtype=np.float32)   # [B,S,D]
    wa = np.asarray(inputs["c_attn_w"], dtype=np.float32)       # [D, 3D]
    ba = np.asarray(inputs["c_attn_b"], dtype=np.float32)       # [3D]
    wpr = np.asarray(inputs["c_proj_w"], dtype=np.float32)      # [D, D]
    bpr = np.asarray(inputs["c_proj_b"], dtype=np.float32)      # [D]

    bf = ml_dtypes.bfloat16
    xT = np.ascontiguousarray(x.reshape(M, D).T).astype(bf)     # [D, M]
    wq, wk, wv_full = wa[:, 0:D], wa[:, D:2 * D], wa[:, 2 * D:3 * D]
    bq, bk, bv_full = ba[0:D], ba[D:2 * D], ba[2 * D:3 * D]

    in_maps = []
    for r in range(NCORES):
        hs = slice(r * HPC * HD, (r + 1) * HPC * HD)   # this core's head dims
        bqk_r = np.concatenate([bq[hs], bk[hs]])
        in_maps.append({
            "xt": xT,
            "wqk": np.ascontiguousarray(
                np.concatenate([wq[:, hs], wk[:, hs]], axis=1)).astype(bf),
            "wv": np.ascontiguousarray(wv_full[:, hs]).astype(bf),
            "wp": np.ascontiguousarray(wpr[:, r * 128:(r + 1) * 128]).astype(bf),
            "bqk": np.ascontiguousarray(bqk_r),
            "biases": np.ascontiguousarray(np.concatenate(
                [bqk_r, bv_full[hs], bpr[r * 128:(r + 1) * 128]])).astype(bf),
        })
    return in_maps


def assemble(results):
    slices = [results[r]["outT"].T.reshape(B, S, 128) for r in range(NCORES)]
    return np.ascontiguousarray(np.concatenate(slices, axis=2).astype(np.float32))


def kernel(**inputs):
    in_maps = make_in_maps(inputs)
    nc = _get_compiled()
    res = run_bass_kernel_spmd(nc, in_maps, core_ids=list(range(NCORES)))
    return assemble(res.results)


if __name__ == "__main__":
    import reference
    inp = reference.setup_inputs()
    out = kernel(**{k: np.asarray(v) for k, v in inp.items()})
    print(out.shape, out.dtype)


# revision 48
# speedup vs baseline: 1.2446x; 1.0662x over previous
"""Trainium2 Bass kernel for ClassicAttention (B=2, S=2048, D=1024, H=16).

Sharding: tensor-parallel over heads across 8 cores (2 heads/core).
  - Host pre-transposes x to x^T [D, M] and pre-casts all matmul operands
    to bf16, so the kernel has no cast / AllGather / DMA-transpose prologue.
  - QKV projection: each core computes Q^T,K^T (d-major) and V (row-major)
    for its 2 heads over all B*S rows straight from x^T in SBUF.  Biases are
    seeded into PSUM with K=1 outer-product matmuls; evacuation is a plain
    DVE copy (tensor_scalar_add from PSUM measured pathologically slow).
  - Attention: transposed-scores formulation S^T[k,q]; both heads share one
    [128,1024] score tile so each k-tile needs a single exp ACTIVATE.  The
    softmax denominator rides row 64 of the AV accumulator via a ones
    column in V.  Per-kt software pipeline: scores(kt+1) is emitted before
    AV(kt) so the exp(kt) runs while the PE does useful work.  Per-q-group
    normalization broadcasts the sums row with a K=1 ones matmul.
  - c_proj: per-(batch, q-group) AllGather of context (8x 128KB), input
    gathers on the gpsimd DMA queue so AllGather waits never block the
    sync queue; each core computes a 128-column slice of the output,
    transposed ([j, B*S]); the host transposes back.
  - Emission order software-pipelines phases: QKV(b1) matmuls interleave
    into attention(b0)'s PE stream, c_proj units into attention(b1).
All matmuls bf16 inputs with fp32 PSUM accumulation.
"""

import numpy as np
import ml_dtypes

import concourse.bass as bass
import concourse.tile as tile
import concourse.mybir as mybir
from concourse import bacc
from concourse.bass_utils import run_bass_kernel_spmd

F32 = mybir.dt.float32
BF16 = mybir.dt.bfloat16

NCORES = 8
B, S, D = 2, 2048, 1024
H, HD = 16, 64
HPC = H // NCORES          # heads per core = 2
M = B * S                  # 4096 rows
ST_B = S // 128            # 16 s-tiles per batch
KCH = D // 128             # 8 contraction chunks
G_PER_B = S // 512         # 4 q-supers per batch
SCALE = 1.0 / (HD ** 0.5)
EXP = mybir.ActivationFunctionType.Exp
DEBUG = False


def build_ir(nc):
    # ---------------- DRAM I/O ----------------
    xt = nc.dram_tensor("xt", [D, M], BF16, kind="ExternalInput").ap()
    wqk = nc.dram_tensor("wqk", [D, 256], BF16, kind="ExternalInput").ap()
    wv = nc.dram_tensor("wv", [D, 128], BF16, kind="ExternalInput").ap()
    wp = nc.dram_tensor("wp", [D, 128], BF16, kind="ExternalInput").ap()
    bqk = nc.dram_tensor("bqk", [256], F32, kind="ExternalInput").ap()
    biases = nc.dram_tensor("biases", [512], BF16, kind="ExternalInput").ap()
    outT = nc.dram_tensor("outT", [128, M], F32, kind="ExternalOutput").ap()

    # causal mask for the diagonal 128-block: mask[k, c] = 1 if c >= k,
    # duplicated for both heads ([128, 2, 128]) so one mul covers a kt tile
    mask_np = (np.arange(128)[None, :] >= np.arange(128)[:, None])
    mask2 = np.stack([mask_np, mask_np], axis=1)
    mask_const = nc.inline_tensor(mask2.astype(ml_dtypes.bfloat16),
                                  "mask_const").ap()

    rg = [list(range(NCORES))]

    dbg = None
    if DEBUG:
        dbg = {
            "qkt": nc.dram_tensor("dbg_qkt", [128, 2, M], BF16,
                                  kind="ExternalOutput").ap(),
            "v": nc.dram_tensor("dbg_v", [128, B * ST_B, 130], BF16,
                                kind="ExternalOutput").ap(),
            "sums": nc.dram_tensor("dbg_sums", [1, 16, 512], F32,
                                   kind="ExternalOutput").ap(),
            "rec": nc.dram_tensor("dbg_rec", [64, 16, 512], F32,
                                  kind="ExternalOutput").ap(),
            "ctx": nc.dram_tensor("dbg_ctx", [128, 8, 512], BF16,
                                  kind="ExternalOutput").ap(),
        }

    with tile.TileContext(nc) as tc:
        _emit(nc, tc, xt, wqk, wv, wp, bqk, biases, outT, mask_const, rg, dbg)
    return nc


def _emit(nc, tc, xt, wqk, wv, wp, bqk, biases, outT, mask_const, rg, dbg=None):
    import contextlib
    es = contextlib.ExitStack()
    with es:
        singles = es.enter_context(tc.tile_pool(name="singles", bufs=1))
        dram = es.enter_context(tc.tile_pool(name="dram", bufs=1, space="DRAM"))

        # ------------- persistent SBUF -------------
        qT = singles.tile([128, M], BF16, tag="qT")
        kT = singles.tile([128, M], BF16, tag="kT")
        v_sb = singles.tile([128, B * ST_B, 130], BF16, tag="v_sb")
        mask_sb = singles.tile([128, 2, 128], BF16, tag="mask_sb")
        nc.sync.dma_start(out=mask_sb, in_=mask_const)
        nc.vector.memset(v_sb, 1.0)                # ones columns pre-set

        # weights (already bf16 from host)
        wqk_sb = singles.tile([128, KCH, 256], BF16, tag="wqk_sb")
        wv_sb = singles.tile([128, KCH, 128], BF16, tag="wv_sb")
        wp_sb = singles.tile([128, KCH, 128], BF16, tag="wp_sb")
        bqk_sb = singles.tile([128, 2], F32, tag="bqk_sb")
        # bias_row: [bqk(256) | bv(128) | bp(128)] bf16, for K=1 seed matmuls
        bias_row = singles.tile([1, 512], BF16, tag="bias_row")
        ones512 = singles.tile([1, 512], BF16, tag="ones512")
        ones_p64 = singles.tile([65, 64], F32, tag="ones_p64")
        nc.vector.memset(ones512, 1.0)
        nc.vector.memset(ones_p64, 1.0)
        nc.sync.dma_start(out=wqk_sb, in_=wqk.rearrange("(c p) j -> p c j", p=128))
        nc.sync.dma_start(out=wv_sb, in_=wv.rearrange("(c p) j -> p c j", p=128))
        nc.sync.dma_start(out=wp_sb, in_=wp.rearrange("(c p) j -> p c j", p=128))
        nc.sync.dma_start(out=bqk_sb, in_=bqk.rearrange("(t p) -> p t", p=128))
        nc.sync.dma_start(out=bias_row, in_=biases.rearrange("(a j) -> a j", a=1))

        # PE warmup: ~25 back-to-back K=1 matmuls engage HAM's full clock
        # before the real instruction stream arrives
        with tc.tile_pool(name="warm_ps", bufs=1, space="PSUM") as warm_ps:
            wt = warm_ps.tile([128, 512], F32)
            for _ in range(15):
                nc.tensor.matmul(wt, lhsT=ones512[:, 0:128], rhs=ones512,
                                 start=True, stop=True)

        # x^T: two [128, 8, 2048] tiles (m-halves), DMA'd in su-major
        # 512-col slices (one DMA per su) so QKV(su) starts early
        xt_r = xt.rearrange("(c p) m -> p c m", p=128)
        xt_h = {h: singles.tile([128, KCH, S], BF16, tag=f"xt_h{h}",
                                name=f"xt_h{h}") for h in range(2)}
        for su in range(8):
            h, o = su // 4, (su % 4) * 512
            nc.sync.dma_start(out=xt_h[h][:, :, o:o + 512],
                              in_=xt_r[:, :, h * S + o:h * S + o + 512])

        def xt_cols(c, m0, m1):
            """slice of x^T chunk c for global columns [m0, m1)"""
            h = m0 // S
            assert m1 <= (h + 1) * S
            return xt_h[h][:, c, m0 - h * S:m1 - h * S]

        # ------------- shared psum pools (8 banks total) -------------
        s_ps = es.enter_context(tc.tile_pool(name="s_ps", bufs=2, space="PSUM"))
        ctx_ps = es.enter_context(tc.tile_pool(name="ctx_ps", bufs=2, space="PSUM"))
        mm_ps = es.enter_context(tc.tile_pool(name="mm_ps", bufs=2, space="PSUM"))

        pt_pool = es.enter_context(tc.tile_pool(name="pt", bufs=4))
        row_pool = es.enter_context(tc.tile_pool(name="row", bufs=2))
        rec_pool = es.enter_context(tc.tile_pool(name="rec", bufs=2))
        cs_pool = es.enter_context(tc.tile_pool(name="cs", bufs=4))
        cg_pool = es.enter_context(tc.tile_pool(name="cg", bufs=3))
        osb = es.enter_context(tc.tile_pool(name="osb", bufs=3))

        # ------------- QKV emitters -------------
        def emit_qk(su, jt, use_act=False):
            """Q^T (jt=0) or K^T (jt=1) for row-super su (512 cols)."""
            dst = qT if jt == 0 else kT
            ps = mm_ps.tile([128, 512], F32, tag="mm")
            if not use_act:   # seed bias via K=1 outer product
                nc.tensor.matmul(ps, lhsT=bias_row[:, jt * 128:(jt + 1) * 128],
                                 rhs=ones512, start=True, stop=False)
            for kc in range(KCH):
                nc.tensor.matmul(
                    ps,
                    lhsT=wqk_sb[:, kc, jt * 128:(jt + 1) * 128],
                    rhs=xt_cols(kc, su * 512, (su + 1) * 512),
                    start=(use_act and kc == 0), stop=(kc == KCH - 1),
                )
            dslice = dst[:, su * 512:(su + 1) * 512]
            if use_act:   # ACT idle in prologue: fused bias-add evacuation
                nc.scalar.add(dslice, ps, bqk_sb[:, jt:jt + 1])
            else:
                nc.vector.tensor_copy(dslice, ps)

        def emit_v(st):
            """V (row-major) for global s-tile st (128 rows)."""
            ps = mm_ps.tile([128, 512], F32, tag="mm")
            nc.tensor.matmul(ps[:, 0:128], lhsT=ones512[:, 0:128],
                             rhs=bias_row[:, 256:384], start=True, stop=False)
            for kc in range(KCH):
                nc.tensor.matmul(
                    ps[:, 0:128],
                    lhsT=xt_cols(kc, st * 128, (st + 1) * 128),
                    rhs=wv_sb[:, kc, :],
                    start=False, stop=(kc == KCH - 1),
                )
            for hl in range(HPC):
                nc.vector.tensor_copy(
                    v_sb[:, st, hl * 65:hl * 65 + 64],
                    ps[:, hl * 64:(hl + 1) * 64])

        # ------------- collective tiles: per (batch, q-group) -------------
        # 8 small AllGathers rather than 4 big ones: each is an inter-core
        # sync point, and frequent syncs keep the cores' pacing drift (and
        # thus per-AG wait time) small
        ctx_local, ctx_all = {}, {}
        for b in range(B):
            for g in range(G_PER_B):
                ctx_local[(b, g)] = dram.tile(
                    [128, 512], BF16, tag=f"ctxl{b}{g}", name=f"ctxl{b}{g}")
                ctx_all[(b, g)] = dram.tile(
                    [NCORES * 128, 512], BF16, addr_space="Shared",
                    tag=f"ctxa{b}{g}", name=f"ctxa{b}{g}")

        # ------------- c_proj emitters (two-phase) -------------
        # phase 1 (emit_cg): issue the 8 gather DMAs on the sync queue --
        # their AllGather wait only blocks later gathers, never the PE.
        # phase 2 (emit_cpmm): the matmuls, popped >=1 q-group later so the
        # gathered data is resident when the in-order PE stream reaches them.
        cg_sets = {}

        def emit_cg(b, g):
            ca = ctx_all[(b, g)]
            # [1024, 512] rank-major rows -> [128, 8, 512] (p, c, m)
            src = bass.AP(tensor=ca.tensor, offset=ca.offset,
                          ap=[[512, 128], [128 * 512, NCORES], [1, 512]])
            cg = cg_pool.tile([128, NCORES, 512], BF16, tag="cg")
            nc.sync.dma_start(out=cg, in_=src)
            cg_sets[(b, g)] = cg

        def emit_cpmm(b, g):
            """output cols [b*S + g*512, +512), transposed [j, m]."""
            cg = cg_sets.pop((b, g))
            ps = mm_ps.tile([128, 512], F32, tag="mm")
            nc.tensor.matmul(ps, lhsT=bias_row[:, 384:512], rhs=ones512,
                             start=True, stop=False)
            for c in range(NCORES):
                nc.tensor.matmul(
                    ps, lhsT=wp_sb[:, c, :], rhs=cg[:, c, :],
                    start=False, stop=(c == NCORES - 1),
                )
            o = osb.tile([128, 512], F32, tag="o")
            nc.vector.tensor_copy(o, ps)
            col = b * S + g * 512
            nc.scalar.dma_start(out=outT[:, col:col + 512], in_=o)

        # ------------- attention -------------
        def emit_attn(b, fill, fill_per_kt, add_after_g=None):
            """Attention for batch b.  Per-kt pipeline: scores(kt+1) is
            emitted before AV(kt).  Pops fill-units between kt steps;
            add_after_g[g] units join the queue only after g's epilogue."""
            for g in range(G_PER_B):
                n_kt = 4 * g + 4
                cps = [ctx_ps.tile([65, 512], F32, tag="ctx", name=f"cps{_hl}")
                       for _hl in range(HPC)]
                q_sl = [qT[hl * 64:(hl + 1) * 64,
                           b * S + g * 512:b * S + (g + 1) * 512]
                        for hl in range(HPC)]
                pend_av = None
                for kt in range(n_kt):
                    qo = max(kt - 4 * g, 0) * 128  # causal trim offset
                    sp = s_ps.tile([128, 2, 512], F32, tag="s")
                    pt = pt_pool.tile([128, 2, 512], BF16, tag="pt")
                    for hl in range(HPC):
                        nc.tensor.matmul(
                            sp[:, hl, qo:512],
                            lhsT=kT[hl * 64:(hl + 1) * 64,
                                    b * S + kt * 128:b * S + (kt + 1) * 128],
                            rhs=q_sl[hl][:, qo:512],
                            start=True, stop=True,
                            tile_position=(64 * hl, 0),
                        )
                    if qo > 0:
                        nc.vector.memset(pt[:, :, 0:qo], 0.0)
                    nc.scalar.activation(pt[:, :, qo:512], sp[:, :, qo:512],
                                         EXP, scale=SCALE)
                    if kt >= 4 * g:   # diagonal block mask, both heads
                        nc.vector.tensor_mul(
                            pt[:, :, qo:qo + 128], pt[:, :, qo:qo + 128],
                            mask_sb)
                    if pend_av is not None:
                        pend_av()
                    def av(kt=kt, pt=pt):
                        for hl in range(HPC):
                            nc.tensor.matmul(
                                cps[hl],
                                lhsT=v_sb[:, b * ST_B + kt,
                                          hl * 65:hl * 65 + 65],
                                rhs=pt[:, hl, :],
                                start=(kt == 0), stop=(kt == n_kt - 1),
                            )
                    pend_av = av
                    for _ in range(fill_per_kt):
                        if fill:
                            fill.pop(0)()
                pend_av()
                # per-g normalize + ctx out: copy the sums row, broadcast it
                # across partitions with a K=1 ones matmul, fast reciprocal,
                # then scale ctx straight out of PSUM
                for hl in range(HPC):
                    row = row_pool.tile([65, 512], F32, tag="row")
                    nc.vector.tensor_copy(row[64:65, :], cps[hl][64:65, :])
                    bc_ps = mm_ps.tile([128, 512], F32, tag="mm")
                    nc.tensor.matmul(bc_ps[0:64, :], lhsT=ones_p64[64:65, :],
                                     rhs=row[64:65, :], start=True, stop=True,
                                     tile_position=(64, 0))
                    rec = rec_pool.tile([64, 512], F32, tag="rec")
                    nc.vector.reciprocal_approx_fast(rec, bc_ps[0:64, :])
                    cs = cs_pool.tile([64, 512], BF16, tag="cs")
                    nc.vector.tensor_mul(cs, cps[hl][0:64, :], rec)
                    nc.gpsimd.dma_start(
                        out=ctx_local[(b, g)][hl * 64:(hl + 1) * 64, :],
                        in_=cs)
                    if dbg is not None:
                        gi = b * 8 + g * 2 + hl
                        nc.gpsimd.dma_start(out=dbg["sums"][:, gi, :],
                                            in_=row[64:65, :])
                        nc.gpsimd.dma_start(out=dbg["rec"][:, gi, :], in_=rec)
                        nc.gpsimd.dma_start(
                            out=dbg["ctx"][hl * 64:(hl + 1) * 64, b * 4 + g, :],
                            in_=cs)
                nc.gpsimd.collective_compute(
                    "AllGather", mybir.AluOpType.bypass, replica_groups=rg,
                    ins=[ctx_local[(b, g)].opt()],
                    outs=[ctx_all[(b, g)].opt()],
                )
                if add_after_g and g in add_after_g:
                    fill.extend(add_after_g[g])
            return fill

        # ------------- choreography -------------
        # minimal QKV prologue for attention(b0) g0 (ACT evacuation: idle)
        emit_qk(0, 0, use_act=True)
        emit_qk(0, 1, use_act=True)
        for st in range(4):
            emit_v(st)

        # rest of QKV b0 (ordered so g deps are met), then QKV b1
        fill = []
        for su in range(1, 8):
            fill.append(lambda su=su: emit_qk(su, 0))
            fill.append(lambda su=su: emit_qk(su, 1))
            for st in range(su * 4, su * 4 + 4):
                fill.append(lambda st=st: emit_v(st))

        cg_u = lambda b, g: (lambda: emit_cg(b, g))
        mm_u = lambda b, g: (lambda: emit_cpmm(b, g))
        after0 = {1: [cg_u(0, 0)], 2: [cg_u(0, 1)], 3: [cg_u(0, 2)]}
        fill = emit_attn(0, fill, 1, add_after_g=after0)
        for f in fill:   # leftovers
            f()

        # attention b1, interleaving c_proj(b0) and c_proj(b1)
        fill2 = [mm_u(0, 0), cg_u(0, 3), mm_u(0, 1)]
        after1 = {0: [mm_u(0, 2), cg_u(1, 0)], 1: [mm_u(0, 3), cg_u(1, 1)],
                  2: [mm_u(1, 0), cg_u(1, 2)], 3: [mm_u(1, 1), cg_u(1, 3)]}
        fill2 = emit_attn(1, fill2, 1, add_after_g=after1)
        for f in fill2:
            f()
        emit_cpmm(1, 2)
        emit_cpmm(1, 3)

        if dbg is not None:
            nc.sync.dma_start(out=dbg["qkt"][:, 0, :], in_=qT)
            nc.sync.dma_start(out=dbg["qkt"][:, 1, :], in_=kT)
            nc.sync.dma_start(out=dbg["v"], in_=v_sb)


_CACHE = {}


def _get_compiled():
    if "nc" not in _CACHE:
        nc = bacc.Bacc("TRN2", target_bir_lowering=False, debug=False,
                       num_devices=NCORES)
        build_ir(nc)
        nc.compile()
        _CACHE["nc"] = nc
    return _CACHE["nc"]


def make_in_maps(inputs):
    x = np.asarray(inputs["hidden_states"], dtype=np.float32)   # [B,S,D]
    wa = np.asarray(inputs["c_attn_w"], dtype=np.float32)       # [D, 3D]
    ba = np.asarray(inputs["c_attn_b"], dtype=np.float32)       # [3D]
    wpr = np.asarray(inputs["c_proj_w"], dtype=np.float32)      # [D, D]
    bpr = np.asarray(inputs["c_proj_b"], dtype=np.float32)      # [D]

    bf = ml_dtypes.bfloat16
    xT = np.ascontiguousarray(x.reshape(M, D).T).astype(bf)     # [D, M]
    wq, wk, wv_full = wa[:, 0:D], wa[:, D:2 * D], wa[:, 2 * D:3 * D]
    bq, bk, bv_full = ba[0:D], ba[D:2 * D], ba[2 * D:3 * D]

    in_maps = []
    for r in range(NCORES):
        hs = slice(r * HPC * HD, (r + 1) * HPC * HD)   # this core's head dims
        bqk_r = np.concatenate([bq[hs], bk[hs]])
        in_maps.append({
            "xt": xT,
            "wqk": np.ascontiguousarray(
                np.concatenate([wq[:, hs], wk[:, hs]], axis=1)).astype(bf),
            "wv": np.ascontiguousarray(wv_full[:, hs]).astype(bf),
            "wp": np.ascontiguousarray(wpr[:, r * 128:(r + 1) * 128]).astype(bf),
            "bqk": np.ascontiguousarray(bqk_r),
            "biases": np.ascontiguousarray(np.concatenate(
                [bqk_r, bv_full[hs], bpr[r * 128:(r + 1) * 128]])).astype(bf),
        })
    return in_maps


def assemble(results):
    slices = [results[r]["outT"].T.reshape(B, S, 128) for r in range(NCORES)]
    return np.ascontiguousarray(np.concatenate(slices, axis=2).astype(np.float32))


def kernel(**inputs):
    in_maps = make_in_maps(inputs)
    nc = _get_compiled()
    res = run_bass_kernel_spmd(nc, in_maps, core_ids=list(range(NCORES)))
    return assemble(res.results)


if __name__ == "__main__":
    import reference
    inp = reference.setup_inputs()
    out = kernel(**{k: np.asarray(v) for k, v in inp.items()})
    print(out.shape, out.dtype)
